# revision 7
# baseline (speedup 1.0000x reference)
"""Single-head attention (B=4, S=4096, E=1024, D=128) on 8 TRN2 NeuronCores.

Sharding: 8 cores = (batch b in 0..3) x (query-half h in 0..1). Each core
computes the attention output for its 2048 queries over the full 4096-key
sequence of its batch. K/V projections are recomputed per core (cheaper than
cross-core exchange). Inputs are pre-transposed on host to xT [E, S] and the
sequence axis is rotated so each core's query rows are columns 0..2047 of its
own xT (softmax is permutation-invariant over keys, so the rotation is free).

Per-core kernel (bf16 matmul operands, fp32 PSUM accumulation):
  x is host-packed to [128, NKB, EC, KB] (partition-major, block-major) so a
  block's DMA descriptors are multi-KB runs; weights to [128, EC, D]. wk leads
  the SCALAR HWDGE ring (wv, wq behind it); block 0 of x is split into 4
  2-chunk pieces on the SYNC ring so the first K-projection matmul can start
  as soon as wk + the first piece land (~10us) instead of waiting for the
  whole block. The tensor engine clock reaches full speed only after ~5us of
  continuous execution (mid p-state = half rate), so junk warmup matmuls (on
  a gpsimd-memset tile, no DVE dependency) keep the PE streaming from
  barrier-release until the first data arrives.
  Rolling schedule: one pass over the 8 projection k-blocks (KT via dense
  512-col streams; V via VT + PE-transposes batched into one PSUM tile; QT
  for blocks 0-3; 1/sqrt(D) folded into Wq on the host), with ALL four
  512-query blocks' attention iterations interleaved between the projection
  parts as their k-chunks/QT become available (strict one-block emission lag,
  <=11 iterations per block). Only the kp>=14 iterations (chunks from the
  last block) remain after the projections, and the per-qb finishes are
  staggered between them.
  Attention iteration (kp, qb) = 2 k-chunks x 512 queries:
           scoresT[k,q] = KTc.T @ QTblk       (PE -> PSUM, per-chunk tile)
           expT = exp(scoresT)                (one ACT pass per chunk)
           oT  += Vc.T @ expT                 (PE, PSUM accum over k)
  Denominator (sum over k of expT), per qb except the last: a 4-level add
  tree reduced to ONE root tile - pairs and quads on the DVE, octs and the
  root on the otherwise-idle Pool engine - then a single ones-matmul per qb
  (emitted in the tail, where the PE is exp-paced and has slack). For the
  LAST qb the tree is bypassed: ones-matmuls accumulate its denominator
  directly in PSUM (shortest possible finish chain). Scores for iteration
  i+1 are emitted before iteration i's AV so the PE covers exp latency.
  Finally oT * reciprocal(den) on DVE, bf16 out, DMA out. For the last qb
  the denominator stop-matmuls are emitted BEFORE its last AV pair so the
  reciprocal overlaps the final AVs.
  No max-subtraction: scores are bounded (|s| <~ 5) so plain exp is safe,
  which removes all flash-attention online-softmax rescaling and lets the
  denominator be accumulated linearly.

PSUM budget (8 banks): o-accumulators x4 + scores x2 (single-chunk tiles,
double-buffered) + projection x2 (double-buffered, shared with the
denominator dtmp/dtl tiles via the same pool tag).
"""

import math
import sys
from contextlib import ExitStack

import numpy as np

if "/opt/trn_rl_repo" not in sys.path:
    sys.path.insert(0, "/opt/trn_rl_repo")

import concourse.bass as bass  # noqa: E402
import concourse.tile as tile  # noqa: E402
from concourse import bacc, mybir  # noqa: E402
from concourse.bass_utils import run_bass_kernel_spmd  # noqa: E402
from concourse.masks import make_identity  # noqa: E402

F32 = mybir.dt.float32
BF16 = mybir.dt.bfloat16

B, S, E, D = 4, 4096, 1024, 128
N_CORES = 8
QH = S // 2  # queries per core


def build_nc(S_kv, S_q, E_, D_, KB=512, QB=512, mm_dt=BF16):
    """Build the per-core Bass program."""
    EC = E_ // 128  # E chunks (contraction)
    NKB = S_kv // KB  # projection k-blocks
    NQB = S_q // QB  # attention q-blocks
    NKC = S_kv // 128  # attention k-chunks
    TPB = KB // 128  # k-chunks per k-block
    NKP = NKC // 2  # attention k-chunk pairs

    nc = bacc.Bacc(
        "TRN2",
        target_bir_lowering=False,
        debug=False,
        enable_asserts=False,
        num_devices=1,
    )
    # x host-packed partition-major, block-major: [128, NKB, EC, KB]
    xq = nc.dram_tensor("xq", [128, NKB, EC, KB], mm_dt, kind="ExternalInput")
    # weights host-packed partition-major: [128, EC, D]
    wq = nc.dram_tensor("wq", [128, EC, D_], mm_dt, kind="ExternalInput")
    wk = nc.dram_tensor("wk", [128, EC, D_], mm_dt, kind="ExternalInput")
    wv = nc.dram_tensor("wv", [128, EC, D_], mm_dt, kind="ExternalInput")
    oT = nc.dram_tensor("oT", [D_, S_q], mm_dt, kind="ExternalOutput")

    with tile.TileContext(nc) as tc, ExitStack() as ctx:
        consts = ctx.enter_context(tc.tile_pool(name="consts", bufs=1))
        persist = ctx.enter_context(tc.tile_pool(name="persist", bufs=1))
        xpool = ctx.enter_context(tc.tile_pool(name="xblk", bufs=3))
        vt_pool = ctx.enter_context(tc.tile_pool(name="vt", bufs=2))
        p_pool = ctx.enter_context(tc.tile_pool(name="pchunk", bufs=10))
        pair_pool = ctx.enter_context(tc.tile_pool(name="pairs", bufs=16))
        o_pool = ctx.enter_context(tc.tile_pool(name="osb", bufs=6))

        # Junk warmup tile: memset on the (idle) Pool engine so the warmup
        # matmuls start right after the preamble barrier, with no DVE dep.
        junk_sb = consts.tile([128, 128], mm_dt, tag="junk_sb")
        nc.gpsimd.memset(junk_sb, 0.0)

        # wk gates the very first matmul: put it FIRST on the scalar HWDGE
        # ring (its engine is otherwise busy with the auto-inserted ACT
        # table load, but the ring runs in parallel); wv/wq follow behind -
        # they are needed only once block 0's VT/QT parts run. The sync
        # ring carries x exclusively, so wk and x piece 0 land in parallel.
        w_sb = {}
        for name, w in (("wk", wk), ("wv", wv), ("wq", wq)):
            w_sb[name] = consts.tile(
                [128, EC, D_], mm_dt, tag=f"w_{name}", name=f"w_{name}"
            )
        nc.scalar.dma_start(out=w_sb["wk"], in_=wk.ap())
        nc.scalar.dma_start(out=w_sb["wv"], in_=wv.ap())
        nc.scalar.dma_start(out=w_sb["wq"], in_=wq.ap())

        ones = consts.tile([128, 128], mm_dt, tag="ones")
        ones_f32 = consts.tile([128, 128], F32, tag="ones_f32")
        nc.vector.memset(ones_f32, 1.0)
        nc.vector.tensor_copy(ones, ones_f32)
        ident = consts.tile([128, 128], mm_dt, tag="ident")
        ident_f32 = consts.tile([128, 128], F32, tag="ident_f32")
        make_identity(nc, ident_f32)
        nc.vector.tensor_copy(ident, ident_f32)

        kt_sb = persist.tile([128, S_kv], mm_dt, tag="kt")  # KT [D, S_kv]
        v_sb = persist.tile([128, NKC, D_], mm_dt, tag="v")  # V chunks [k128, D]
        qt_sb = persist.tile([128, S_q], mm_dt, tag="qt")  # QT [D, S_q]

        # p-state warmup: the tensor engine reaches full clock only after
        # ~5us of CONTINUOUS execution; junk matmuls keep the PE streaming
        # from barrier-time until the first real data arrives (~10us).
        with tc.tile_pool(name="ps_warm", bufs=1, space="PSUM") as pswp:
            wt = pswp.tile([128, 4, 128], F32, tag="warm", name="warm")
            for wi in range(30):
                nc.tensor.matmul(
                    wt[:, wi % 4, :], lhsT=junk_sb, rhs=junk_sb, start=True, stop=True
                )

        def proj_block(kb, ps_proj):
            """Emit projection work for k-block kb as a list of closures so the
            caller can interleave attention iterations between the parts."""
            xblk = xpool.tile([128, EC, KB], mm_dt, tag="xblk", name=f"xblk_{kb}")
            if kb == 0:
                # block 0 in 4 two-chunk pieces: the first K-proj matmul
                # starts once wk + piece 0 land; later pieces stream in
                # while the (still mid-p-state) PE consumes earlier chunks.
                for pi in range(4):
                    nc.sync.dma_start(
                        out=xblk[:, 2 * pi : 2 * pi + 2, :],
                        in_=xq.ap()[:, 0, 2 * pi : 2 * pi + 2, :],
                    )
            else:
                cpd = 4
                for di in range(0, EC, cpd):
                    nc.sync.dma_start(
                        out=xblk[:, di : di + cpd, :],
                        in_=xq.ap()[:, kb, di : di + cpd, :],
                    )

            def part_kt():
                ps_kt = ps_proj.tile([128, KB], F32, tag="ps_proj", name=f"ps_kt_{kb}")
                for c in range(EC):
                    nc.tensor.matmul(
                        ps_kt,
                        lhsT=w_sb["wk"][:, c, :],
                        rhs=xblk[:, c, :],
                        start=(c == 0),
                        stop=(c == EC - 1),
                    )
                nc.vector.tensor_copy(kt_sb[:, kb * KB : (kb + 1) * KB], ps_kt)

            vt_box = {}

            def part_vt():
                # VT[D, KB] via dense 512-col streams + DVE cast to bf16
                ps_vt = ps_proj.tile([128, KB], F32, tag="ps_proj", name=f"ps_vt_{kb}")
                for c in range(EC):
                    nc.tensor.matmul(
                        ps_vt,
                        lhsT=w_sb["wv"][:, c, :],
                        rhs=xblk[:, c, :],
                        start=(c == 0),
                        stop=(c == EC - 1),
                    )
                vt_tmp = vt_pool.tile([128, KB], mm_dt, tag="vt_tmp", name=f"vt_{kb}")
                nc.vector.tensor_copy(vt_tmp, ps_vt)
                vt_box["t"] = vt_tmp

            def part_vtr():
                # PE-transpose all TPB tiles into ONE [128, TPB, 128] PSUM
                # tile + single DVE cast out. Emitted as a SEPARATE part so
                # interleaved attention iterations sit between the vt cast
                # and the transposes that read it - otherwise the PE idles
                # ~0.5us per block waiting for the cast.
                vt_tmp = vt_box["t"]
                ps_tr = ps_proj.tile(
                    [128, TPB, 128], mm_dt, tag="ps_proj", name=f"ps_tr_{kb}"
                )
                for t_ in range(TPB):
                    nc.tensor.transpose(
                        ps_tr[:, t_, :], vt_tmp[:, t_ * 128 : (t_ + 1) * 128], ident
                    )
                nc.vector.tensor_copy(
                    v_sb[:, kb * TPB : (kb + 1) * TPB, :], ps_tr
                )

            def part_qt():
                if kb * KB >= S_q:
                    return
                qw = min(KB, S_q - kb * KB)
                ps_qt = ps_proj.tile([128, KB], F32, tag="ps_proj", name=f"ps_qt_{kb}")
                for c in range(EC):
                    nc.tensor.matmul(
                        ps_qt[:, :qw],
                        lhsT=w_sb["wq"][:, c, :],
                        rhs=xblk[:, c, :qw],
                        start=(c == 0),
                        stop=(c == EC - 1),
                    )
                nc.vector.tensor_copy(qt_sb[:, kb * KB : kb * KB + qw], ps_qt[:, :qw])

            return [part_kt, part_vt, part_vtr, part_qt]

        class AttnEmitter:
            """Attention over a fixed qb set. Iterations (kp, qb) are fed in
            order; scores are emitted 1 ahead. Denominators: per qb a 4-level
            add tree to ONE root (pairs+quads on DVE, octs+root on Pool) and
            a single tail ones-matmul; the LAST qb bypasses the tree and
            accumulates directly in PSUM via ones-matmuls."""

            def __init__(self, qbs, ps_s_pool, ps_od, dtmp_pool):
                self.qbs = qbs
                self.ps_s_pool = ps_s_pool
                self.ps_od = ps_od
                self.dtmp_pool = dtmp_pool
                self.ps_o = {}
                self.dtmp = {}  # per-qb PSUM denominator tile
                self.root = {}  # per-qb fully-reduced [128, QB] denominator
                for qb in qbs:
                    self.ps_o[qb] = ps_od.tile(
                        [128, QB], F32, tag="ps_od", name=f"ps_o_{qb}"
                    )
                # tree levels per qb (non-last): pair->quad->oct->hex->root
                self.held = {
                    qb: ([None, None] if qb == qbs[-1] else [None] * 4)
                    for qb in qbs
                }
                self.lvl_eng = [nc.vector, nc.gpsimd, nc.gpsimd, nc.gpsimd]
                self.idx = 0
                self.pending = None
                self.pending_it = None
                self.tail_dtmp = {}
                self.tail_n = {}
                # the LAST qb's denominator goes entirely into its tail
                # PSUM accumulation (partials held in SBUF until then)
                self.last_qb = qbs[-1]
                self.hold_parts = []

            def _scores(self, it):
                kp, qb = it
                tiles = []
                for j in range(2):
                    kc = 2 * kp + j
                    ps_s = self.ps_s_pool.tile(
                        [128, QB], F32, tag="ps_s", name=f"ps_s_{kp}_{qb}_{j}"
                    )
                    nc.tensor.matmul(
                        ps_s,
                        lhsT=kt_sb[:, kc * 128 : (kc + 1) * 128],
                        rhs=qt_sb[:, qb * QB : (qb + 1) * QB],
                        start=True,
                        stop=True,
                    )
                    tiles.append(ps_s)
                return tiles

            def step(self, it, next_it):
                if self.pending is None:
                    self.pending = self._scores(it)
                    self.pending_it = it
                assert self.pending_it == it
                ps_s = self.pending
                if next_it is not None:
                    self.pending = self._scores(next_it)
                    self.pending_it = next_it
                else:
                    self.pending = None
                kp, qb = it
                p_sb = []
                for j in range(2):
                    p_j = p_pool.tile(
                        [128, QB], mm_dt, tag="p_sb", name=f"p_sb_{kp}_{qb}_{j}"
                    )
                    nc.scalar.activation(
                        p_j, ps_s[j], mybir.ActivationFunctionType.Exp
                    )
                    p_sb.append(p_j)
                is_last_tail = qb == self.last_qb and kp >= NKP - 2
                if not (is_last_tail and kp == NKP - 1):
                    for j in range(2):
                        kc = 2 * kp + j
                        nc.tensor.matmul(
                            self.ps_o[qb],
                            lhsT=v_sb[:, kc, :],
                            rhs=p_sb[j],
                            start=(kp == 0 and j == 0),
                            stop=(kp == NKP - 1 and j == 1),
                        )
                if is_last_tail:
                    # last qb's tail: bypass the tree; accumulate the
                    # remaining denominator directly in PSUM via ones-matmuls
                    if qb not in self.tail_dtmp:
                        dtl = self.dtmp_pool.tile(
                            [128, QB], F32, tag="ps_proj", name=f"dtl_{qb}"
                        )
                        self.tail_dtmp[qb] = dtl
                        self.tail_n[qb] = 0
                        for part in self.hold_parts:
                            nc.tensor.matmul(
                                dtl,
                                lhsT=ones,
                                rhs=part,
                                start=(self.tail_n[qb] == 0),
                                stop=False,
                            )
                            self.tail_n[qb] += 1
                        self.hold_parts = []
                        lvl = self.held[qb]
                        for li in range(len(lvl)):
                            if lvl[li] is not None:
                                nc.tensor.matmul(
                                    dtl,
                                    lhsT=ones,
                                    rhs=lvl[li],
                                    start=(self.tail_n[qb] == 0),
                                    stop=False,
                                )
                                self.tail_n[qb] += 1
                                lvl[li] = None
                    dtl = self.tail_dtmp[qb]
                    for j in range(2):
                        nc.tensor.matmul(
                            dtl,
                            lhsT=ones,
                            rhs=p_sb[j],
                            start=(self.tail_n[qb] == 0),
                            stop=(kp == NKP - 1 and j == 1),
                        )
                        self.tail_n[qb] += 1
                    if kp == NKP - 1:
                        # emit the last AV pair AFTER the denominator stop so
                        # the reciprocal (which reads dtl) overlaps these AVs
                        for j in range(2):
                            kc = 2 * kp + j
                            nc.tensor.matmul(
                                self.ps_o[qb],
                                lhsT=v_sb[:, kc, :],
                                rhs=p_sb[j],
                                start=False,
                                stop=(j == 1),
                            )
                else:
                    pair = pair_pool.tile(
                        [128, QB], mm_dt, tag="pair", name=f"pair_{kp}_{qb}"
                    )
                    nc.vector.tensor_add(pair, p_sb[0], p_sb[1])
                    lvl = self.held[qb]
                    cur = pair
                    placed = False
                    for li in range(len(lvl)):
                        if lvl[li] is None:
                            lvl[li] = cur
                            placed = True
                            break
                        nxt = pair_pool.tile(
                            [128, QB], mm_dt, tag=f"red{li}",
                            bufs=(8 if li < 2 else 4),
                            name=f"red{li}_{kp}_{qb}",
                        )
                        self.lvl_eng[li].tensor_add(nxt, lvl[li], cur)
                        lvl[li] = None
                        cur = nxt
                    if not placed:
                        if qb == self.last_qb:
                            self.hold_parts.append(cur)
                        else:
                            # cur is the fully-reduced root (32 chunks)
                            self.root[qb] = cur
                self.idx += 1

            def finish_den(self, qb):
                """Complete a non-last qb's denominator: one ones-matmul on
                its tree root (NKP is a power of two, so the cascade fully
                combined and the held levels are empty)."""
                assert all(t is None for t in self.held[qb])
                self.dtmp[qb] = self.dtmp_pool.tile(
                    [128, QB], F32, tag="ps_proj", name=f"dtmp_{qb}"
                )
                nc.tensor.matmul(
                    self.dtmp[qb],
                    lhsT=ones,
                    rhs=self.root[qb],
                    start=True,
                    stop=True,
                )

            def finish_out(self, qb):
                d_src = (
                    self.tail_dtmp[qb] if qb == self.last_qb else self.dtmp[qb]
                )
                rec = o_pool.tile([128, QB], F32, tag="rec")
                nc.vector.reciprocal_approx_fast(out=rec, in_=d_src)
                o_sb = o_pool.tile([128, QB], mm_dt, tag="o_sb")
                nc.vector.tensor_mul(o_sb, self.ps_o[qb], rec)
                eng = nc.sync if qb % 2 == 0 else nc.scalar
                eng.dma_start(
                    out=oT.ap()[:, qb * QB : (qb + 1) * QB],
                    in_=o_sb,
                )

        # ---- rolling schedule ----
        # All qbs' attention is interleaved into the projection stream.
        # Iteration (kp, qb) becomes available at block kb when its k-chunks
        # and QT block were emitted at least one block earlier (strict lag:
        # kp < TPB*kb/2, qb*QB < kb*KB). Only the kp >= NKP-2 iterations
        # (whose chunks come from the last block) plus the per-qb finish
        # chains remain after the last projection block, and the finishes
        # are staggered so each overlaps the remaining qbs' iterations.
        qbs = tuple(range(NQB))
        CAP = 11  # per-block iteration cap (keeps ACT below the block wall)

        # precompute per-block take lists + the post-projection tail so
        # step() can see next_it across the whole sequence
        next_kp = {qb: 0 for qb in qbs}
        takes = []  # per-block iteration lists
        for kb in range(NKB):
            kp_avail = min((TPB * kb) // 2, NKP)
            take = []
            progress = True
            while len(take) < CAP and progress:
                progress = False
                for qb in qbs:
                    if qb * QB < kb * KB and next_kp[qb] < kp_avail:
                        take.append((next_kp[qb], qb))
                        next_kp[qb] += 1
                        progress = True
                        if len(take) >= CAP:
                            break
            takes.append(take)
        tail = []  # (it or None, finish_qb or None)
        for qb in qbs:
            while next_kp[qb] < NKP:
                tail.append((next_kp[qb], qb))
                next_kp[qb] += 1
        iter_seq = [it for take in takes for it in take] + tail
        assert len(iter_seq) == NKP * NQB
        it_next = {
            it: (iter_seq[i + 1] if i + 1 < len(iter_seq) else None)
            for i, it in enumerate(iter_seq)
        }

        with tc.tile_pool(name="ps_o", bufs=NQB, space="PSUM") as ps_o_pool, \
             tc.tile_pool(name="ps_s", bufs=2, space="PSUM") as ps_sF, \
             tc.tile_pool(name="ps_pj", bufs=2, space="PSUM") as ps_pj:
            att = AttnEmitter(qbs, ps_sF, ps_o_pool, dtmp_pool=ps_pj)
            for kb in range(NKB):
                parts = proj_block(kb, ps_pj)
                take = takes[kb]
                nparts = len(parts)
                per = (len(take) + nparts) // (nparts + 1)
                ti = 0
                for pi, part in enumerate(parts):
                    part()
                    if kb == 0 and pi == 1:
                        # block 0 has no attention iterations to cover the
                        # vt-cast -> transposes seam; fill the ~0.8us PE
                        # wait with junk matmuls into the (still unused)
                        # scores pool
                        junk = ps_sF.tile(
                            [128, QB], F32, tag="ps_s", name="junk0"
                        )
                        for ji in range(14):
                            nc.tensor.matmul(
                                junk[:, :128], lhsT=junk_sb, rhs=junk_sb,
                                start=True, stop=True,
                            )
                    for it in take[ti : ti + per]:
                        att.step(it, it_next[it])
                    ti += per
                for it in take[ti:]:
                    att.step(it, it_next[it])
            # tail: finish_den(q) right after q's last step (its tree inputs
            # are already complete); finish_out(q) one iteration later so
            # the reciprocal doesn't queue behind it
            for it in tail:
                kp, qb = it
                if kp == NKP - 2 and qb > 0:
                    att.finish_out(qb - 1)
                att.step(it, it_next[it])
                if kp == NKP - 1 and qb != att.last_qb:
                    att.finish_den(qb)
            att.finish_out(qbs[-1])

    nc.compile()
    return nc


_NC_CACHE = {}


def _get_nc(key, *args, **kwargs):
    if key not in _NC_CACHE:
        _NC_CACHE[key] = build_nc(*args, **kwargs)
    return _NC_CACHE[key]


def run_cores(nc, in_maps, **kwargs):
    core_ids = list(range(len(in_maps)))
    return run_bass_kernel_spmd(nc, in_maps, core_ids=core_ids, **kwargs)


def run_cores_profiled(nc, in_maps, trace_cores=(0,)):
    """Run via PJRT with NRT profiling (the antenv hook is missing in this
    container, so drive the ctypes profile start/stop directly)."""
    import glob
    import tempfile

    import gauge.profiler
    from concourse import bass2jax
    from concourse._compat import FishPath
    from trn_agent_boot.trn_boot import _ntff_profile_via_ctypes

    hook = _ntff_profile_via_ctypes("/opt/axon/libaxon_pjrt.so")
    neff_dir = tempfile.mkdtemp(prefix="attn_prof_")
    with hook(neff_dir, list(trace_cores)):
        results = bass2jax.run_bass_via_pjrt(nc, in_maps, n_cores=len(in_maps))
    ntffs = glob.glob(neff_dir + "/*_body*.ntff")
    if not ntffs:
        print("WARNING: no NTFFs captured in", neff_dir)
        return results, None, None
    profile = gauge.profiler.Profile(
        profile_path=FishPath(neff_dir),
        kernel_dev_mode=True,
        profile_on_exit=False,
        bass_kernel=nc.m,
        offline_processing=True,
        fname="*_body*",
        metadata={"artifacts_path": neff_dir},
    )
    prs = profile.to_perfetto(model_index=tuple(trace_cores))
    exec_ns = max(pr.exec_time_ns for pr in prs)
    return results, exec_ns, prs


def _cvt(a):
    import ml_dtypes

    return np.ascontiguousarray(a).astype(ml_dtypes.bfloat16)


def _pack_w(w):
    """[E, D] -> partition-major [128, EC, D] so DMA descriptors are 2KB runs."""
    E_, D_ = w.shape
    return np.ascontiguousarray(w.reshape(E_ // 128, 128, D_).transpose(1, 0, 2))


def _pack_x(xT, KB=512):
    """xT [E, S] -> [128, NKB, EC, KB] partition-major block-major so each
    block's DMA has multi-KB contiguous runs per partition."""
    E_, S_ = xT.shape
    EC = E_ // 128
    NKB = S_ // KB
    # xT[c*128+p, kb*KB+s] -> out[p, kb, c, s]
    return np.ascontiguousarray(
        xT.reshape(EC, 128, NKB, KB).transpose(1, 2, 0, 3)
    )


def kernel(x, Wq, Wk, Wv, _trace=False, _trace_cores=(0,)):
    x = np.asarray(x, dtype=np.float32)
    scale = 1.0 / math.sqrt(Wq.shape[1])
    wq_s = _cvt(_pack_w(np.asarray(Wq, np.float32) * scale))
    wk_ = _cvt(_pack_w(np.asarray(Wk, np.float32)))
    wv_ = _cvt(_pack_w(np.asarray(Wv, np.float32)))

    nc = _get_nc("full_bf16", S, QH, E, D, mm_dt=BF16)
    in_maps = []
    for c in range(N_CORES):
        b, h = divmod(c, 2)
        xb = x[b]
        if h == 0:
            xr = xb
        else:
            xr = np.concatenate([xb[QH:], xb[:QH]], axis=0)
        in_maps.append(
            {
                "xq": _cvt(_pack_x(xr.T)),
                "wq": wq_s,
                "wk": wk_,
                "wv": wv_,
            }
        )
    if _trace:
        results, exec_ns, prs = run_cores_profiled(nc, in_maps, trace_cores=_trace_cores)
        kernel.last_exec_time_ns = exec_ns
        kernel.last_prs = prs
    else:
        results = run_cores(nc, in_maps).results
    out = np.empty((B, S, D), dtype=np.float32)
    for c in range(N_CORES):
        b, h = divmod(c, 2)
        out[b, h * QH : (h + 1) * QH, :] = (
            np.asarray(results[c]["oT"]).astype(np.float32).T
        )
    return out
